# revision 1
# baseline (speedup 1.0000x reference)
"""Bahdanau attention on 8 Trainium2 cores (Bass/Tile), data-parallel over B.

reference (per batch b, all shapes full):
    hp  = hidden[0] @ W_h.T + b_h                    # (B, H)
    ep  = einsum('tbh,gh->btg', enc, W_e) + b_e      # (B, T, H)
    en  = tanh(hp[:, None, :] + ep)                  # (B, T, H)
    sc  = en @ v                                     # (B, T)
    out = softmax(sc, -1)[:, None, :]                # (B, 1, T)

Sharding: B=32 split 4-per-core across 8 cores; W_h/W_e/b/v replicated.
Per-core kernel layout: tokens of one batch are processed in groups of 512;
enc tiles are PE-transposed to put H on partitions; ep accumulates over
8 h-chunks in PSUM as [g=128, tok=512] via fp32r matmuls (full PE rate,
~tf32 accuracy); ACT applies tanh with the per-partition bias
hp^T[:, b] + b_h + b_e; a [128x4] fp32r matmul against v4 (v in column b,
zeros elsewhere) reduces over g so batch b's scores land on PSUM partition
b; SBUF-to-SBUF DMA parks each batch's score row at partition 32*b, and
that batch's softmax + output DMA run incrementally as soon as its last
token group finishes, hiding the tail inside the main loop.
"""

import sys
from contextlib import ExitStack

import numpy as np

try:
    import concourse  # noqa: F401
except ImportError:  # pragma: no cover
    sys.path.insert(0, "/opt/trn_rl_repo")

import concourse.tile as tile
from concourse import bacc, mybir
from concourse.bass import ts
from concourse.bass_utils import run_bass_kernel_spmd
from concourse.masks import make_identity

H = 1024
T = 2048
B = 32
NCORES = 8
BC = B // NCORES          # batches per core
HC = H // 128             # h chunks
GC = H // 128             # g chunks
TOK = 512                 # tokens per group (one batch each)
SUB = TOK // 128          # 128-token subtiles per group
NGRP_PER_B = T // TOK
NGRP = BC * NGRP_PER_B

F32 = mybir.dt.float32
F32R = mybir.dt.float32r
AF = mybir.ActivationFunctionType
AX = mybir.AxisListType


def build_kernel_nc(reps=1):
    nc = bacc.Bacc(
        "TRN2",
        target_bir_lowering=False,
        debug=False,
        enable_asserts=False,
        num_devices=NCORES,
    )
    enc = nc.dram_tensor("enc", [T, BC, H], F32, kind="ExternalInput").ap()
    hid = nc.dram_tensor("hid", [BC, H], F32, kind="ExternalInput").ap()
    w_e = nc.dram_tensor("W_e", [H, H], F32, kind="ExternalInput").ap()
    w_h = nc.dram_tensor("W_h", [H, H], F32, kind="ExternalInput").ap()
    b_h = nc.dram_tensor("b_h", [H], F32, kind="ExternalInput").ap()
    b_e = nc.dram_tensor("b_e", [H], F32, kind="ExternalInput").ap()
    v = nc.dram_tensor("v", [H], F32, kind="ExternalInput").ap()
    out = nc.dram_tensor("out", [BC, T], F32, kind="ExternalOutput").ap()

    with tile.TileContext(nc) as tc:
        _kernel_body(tc, enc, hid, w_e, w_h, b_h, b_e, v, out, reps=reps)
    nc.compile()
    return nc


def _kernel_body(tc, enc, hid, w_e, w_h, b_h, b_e, v, out, reps=1):
    nc = tc.nc
    with ExitStack() as ctx:
        singles = ctx.enter_context(tc.tile_pool(name="singles", bufs=1))
        enc_pool = ctx.enter_context(tc.tile_pool(name="enc_nat", bufs=2 * SUB))
        encT_pool = ctx.enter_context(tc.tile_pool(name="encT", bufs=2))
        energy_pool = ctx.enter_context(tc.tile_pool(name="energy", bufs=3))
        scrow_pool = ctx.enter_context(tc.tile_pool(name="scrow", bufs=2))
        trps_pool = ctx.enter_context(
            tc.tile_pool(name="trps", bufs=3, space="PSUM")
        )
        ep_pool = ctx.enter_context(tc.tile_pool(name="epps", bufs=3, space="PSUM"))
        sc_pool = ctx.enter_context(tc.tile_pool(name="scps", bufs=2, space="PSUM"))

        identity = singles.tile([128, 128], F32)
        make_identity(nc, identity[:])

        # ---- persistent SBUF tensors -------------------------------------
        WeT = singles.tile([128, HC, H], F32R)     # WeT[h, hc, g] = W_e[g, 128*hc+h]
        WhT = singles.tile([128, HC, H], F32)
        hidT = singles.tile([128, HC, BC], F32)    # hidT[h, hc, b] = hid[b, 128*hc+h]
        bias_all = singles.tile([128, GC, BC], F32)  # hp^T + b_h + b_e
        v_sb = singles.tile([128, GC], F32)        # v[gc*128+p] at [p, gc]
        # v4[:, gc, b, :] is a [128, BC] stationary operand whose column b
        # holds the v chunk and the rest are zero -> batch b's scores land
        # on PSUM partition b (fp32r matmuls require dst partition 0).
        v4f = singles.tile([128, GC, BC, BC], F32)
        v4 = singles.tile([128, GC, BC, BC], F32R)
        bsum = singles.tile([128, GC], F32)        # (b_h + b_e) chunked
        # batch b's scores live on partition 32*b so per-batch softmax can
        # run as soon as that batch's groups finish (engine ops only accept
        # partition bases 0/32/64/96; DMA scatters the rows there)
        scores = singles.tile([128, T], F32)
        probs = singles.tile([128, T], F32)
        negmax = singles.tile([128, 1], F32)
        sums = singles.tile([128, 1], F32)
        rsum = singles.tile([128, 1], F32)

        # ---- stage 0: weights transpose + hp + biases --------------------
        bh_sb = singles.tile([128, GC], F32)
        be_sb = singles.tile([128, GC], F32)
        nc.sync.dma_start(out=bh_sb[:], in_=b_h.rearrange("(c p) -> p c", p=128))
        nc.sync.dma_start(out=be_sb[:], in_=b_e.rearrange("(c p) -> p c", p=128))
        nc.sync.dma_start(out=v_sb[:], in_=v.rearrange("(c p) -> p c", p=128))
        nc.vector.tensor_add(bsum[:], bh_sb[:], be_sb[:])
        nc.gpsimd.memset(v4f[:], 0.0)
        for b in range(BC):
            for gc in range(GC):
                nc.vector.tensor_copy(v4f[:, gc, b, b : b + 1], v_sb[:, gc : gc + 1])
        nc.vector.tensor_copy(v4[:], v4f[:])

        with tc.tile_pool(name="stage0", bufs=4) as wload:
            for w_src, w_dst in ((w_e, WeT), (w_h, WhT)):
                for gc in range(GC):
                    wn = wload.tile([128, H], F32, tag="wn")
                    nc.sync.dma_start(out=wn[:], in_=w_src[ts(gc, 128), :])
                    for hc in range(HC):
                        tp = trps_pool.tile([128, 128], F32, tag="tr")
                        nc.tensor.transpose(tp[:], wn[:, ts(hc, 128)], identity[:])
                        nc.vector.tensor_copy(w_dst[:, hc, ts(gc, 128)], tp[:])

            hid_nat = wload.tile([BC, H], F32, tag="hid")
            nc.sync.dma_start(out=hid_nat[:], in_=hid[:, :])
            for hc in range(HC):
                tph = trps_pool.tile([128, BC], F32, tag="tr")
                nc.tensor.transpose(
                    tph[:], hid_nat[:, ts(hc, 128)], identity[0:BC, 0:BC]
                )
                nc.vector.tensor_copy(hidT[:, hc, :], tph[:])

            # hp^T[g, b] accumulated over h chunks (fp32, tiny N)
            for gc in range(GC):
                hp_ps = trps_pool.tile([128, BC], F32, tag="tr")
                for hc in range(HC):
                    nc.tensor.matmul(
                        hp_ps[:],
                        WhT[:, hc, ts(gc, 128)],
                        hidT[:, hc, :],
                        start=(hc == 0),
                        stop=(hc == HC - 1),
                    )
                nc.vector.tensor_scalar(
                    out=bias_all[:, gc, :],
                    in0=hp_ps[:],
                    scalar1=bsum[:, gc : gc + 1],
                    scalar2=None,
                    op0=mybir.AluOpType.add,
                )

        # ---- main loop: 16 groups of 512 tokens --------------------------
        # Software-pipelined so the in-order PE queue never waits on ACT:
        #   iteration g emits: DMA(g+2), transposes(g+1), ep/sc chain(g)
        # with sc(gc-1) emitted after ep(gc) so tanh(gc-1) is long done.
        n_total = reps * NGRP

        def issue_load(grp):
            g = grp % NGRP
            b = g // NGRP_PER_B
            t0 = (g % NGRP_PER_B) * TOK
            en_nat = []
            for s in range(SUB):
                en = enc_pool.tile([128, H], F32, tag="en")
                nc.sync.dma_start(
                    out=en[:], in_=enc[t0 + s * 128 : t0 + (s + 1) * 128, b, :]
                )
                en_nat.append(en)
            return en_nat

        def issue_transposes(en_nat):
            encT = encT_pool.tile([128, HC, TOK], F32R)
            for hc in range(HC):
                tp = trps_pool.tile([128, TOK], F32, tag="tr")
                for s in range(SUB):
                    nc.tensor.transpose(
                        tp[:, ts(s, 128)], en_nat[s][:, ts(hc, 128)], identity[:]
                    )
                nc.vector.tensor_copy(encT[:, hc, :], tp[:])
            return encT

        loads = [issue_load(0), issue_load(1)]
        encT_cur = issue_transposes(loads[0])
        carry = None  # deferred final sc-mm of the previous group

        def softmax_b(b):
            r = slice(32 * b, 32 * b + 1)
            nc.vector.tensor_reduce(
                out=negmax[r], in_=scores[r, :], axis=AX.X,
                op=mybir.AluOpType.max, negate=True,
            )
            nc.scalar.activation(
                out=probs[r, :], in_=scores[r, :], func=AF.Exp,
                bias=negmax[r], scale=1.0, accum_out=sums[r],
            )
            nc.vector.reciprocal(out=rsum[r], in_=sums[r])
            nc.vector.tensor_scalar_mul(probs[r, :], probs[r, :], rsum[r])
            nc.sync.dma_start(out=out[b : b + 1, :], in_=probs[r, :])

        def flush_carry(c):
            c_sc_ps, c_gc, c_energy, c_b, c_t0 = c
            nc.tensor.matmul(
                c_sc_ps[:], v4[:, c_gc, c_b, :], c_energy[:],
                start=False, stop=True,
            )
            sc_sb = scrow_pool.tile([BC, TOK], F32)
            nc.vector.tensor_copy(sc_sb[:], c_sc_ps[:])
            nc.sync.dma_start(
                out=scores[32 * c_b : 32 * c_b + 1, c_t0 : c_t0 + TOK],
                in_=sc_sb[c_b : c_b + 1, :],
            )
            if c_t0 == T - TOK:
                softmax_b(c_b)

        for grp in range(n_total):
            g = grp % NGRP
            b = g // NGRP_PER_B
            t0 = (g % NGRP_PER_B) * TOK

            if grp + 2 < n_total:
                loads.append(issue_load(grp + 2))
            encT_next = None
            if grp + 1 < n_total:
                encT_next = issue_transposes(loads[grp + 1])
            if carry is not None:
                flush_carry(carry)
                carry = None

            sc_ps = sc_pool.tile([BC, TOK], F32)
            pending = None
            for gc in range(GC):
                ep_ps = ep_pool.tile([128, TOK], F32)
                for hc in range(HC):
                    nc.tensor.matmul(
                        ep_ps[:],
                        WeT[:, hc, ts(gc, 128)],
                        encT_cur[:, hc, :],
                        start=(hc == 0),
                        stop=(hc == HC - 1),
                    )
                if pending is not None:
                    pc, penergy = pending
                    nc.tensor.matmul(
                        sc_ps[:], v4[:, pc, b, :], penergy[:],
                        start=(pc == 0), stop=False,
                    )
                energy = energy_pool.tile([128, TOK], F32R)
                nc.scalar.activation(
                    out=energy[:],
                    in_=ep_ps[:],
                    func=AF.Tanh,
                    bias=bias_all[:, gc, b : b + 1],
                    scale=1.0,
                )
                pending = (gc, energy)
            pc, penergy = pending
            carry = (sc_ps, pc, penergy, b, t0)
            encT_cur = encT_next

        flush_carry(carry)


_NC_CACHE = None


def _get_nc():
    global _NC_CACHE
    if _NC_CACHE is None:
        _NC_CACHE = build_kernel_nc()
    return _NC_CACHE


def make_in_maps(hidden, encoder_outputs, W_h, b_h, W_e, b_e, v):
    hidden = np.asarray(hidden, dtype=np.float32)
    enc = np.asarray(encoder_outputs, dtype=np.float32)
    W_h = np.ascontiguousarray(np.asarray(W_h, dtype=np.float32))
    W_e = np.ascontiguousarray(np.asarray(W_e, dtype=np.float32))
    b_h = np.ascontiguousarray(np.asarray(b_h, dtype=np.float32))
    b_e = np.ascontiguousarray(np.asarray(b_e, dtype=np.float32))
    v = np.ascontiguousarray(np.asarray(v, dtype=np.float32))
    hid0 = hidden.reshape(B, H)
    in_maps = []
    for c in range(NCORES):
        in_maps.append(
            {
                "enc": np.ascontiguousarray(enc[:, c * BC : (c + 1) * BC, :]),
                "hid": np.ascontiguousarray(hid0[c * BC : (c + 1) * BC, :]),
                "W_e": W_e,
                "W_h": W_h,
                "b_h": b_h,
                "b_e": b_e,
                "v": v,
            }
        )
    return in_maps


def kernel(hidden, encoder_outputs, W_h, b_h, W_e, b_e, v):
    nc = _get_nc()
    in_maps = make_in_maps(hidden, encoder_outputs, W_h, b_h, W_e, b_e, v)
    res = run_bass_kernel_spmd(nc, in_maps, list(range(NCORES)))
    full = np.concatenate([res.results[c]["out"] for c in range(NCORES)], axis=0)
    return full[:, None, :].astype(np.float32)



# revision 2
# speedup vs baseline: 1.1242x; 1.1242x over previous
"""Bahdanau attention on 8 Trainium2 cores (Bass/Tile), data-parallel over B.

reference (per batch b, all shapes full):
    hp  = hidden[0] @ W_h.T + b_h                    # (B, H)
    ep  = einsum('tbh,gh->btg', enc, W_e) + b_e      # (B, T, H)
    en  = tanh(hp[:, None, :] + ep)                  # (B, T, H)
    sc  = en @ v                                     # (B, T)
    out = softmax(sc, -1)[:, None, :]                # (B, 1, T)

Sharding: B=32 split 4-per-core across 8 cores; weights replicated.

v2 layout: host stages enc as bf16 (halves HBM traffic) and ships the
weights pre-transposed (W^T, layout-only) in bf16.  Each 512-token group
of one batch is brought into SBUF already transposed to [h=128, hc, tok]
by a single XBAR dma_start_transpose straight from DRAM, so the PE runs
nothing but the ep/score matmuls (bf16, fp32 PSUM accumulation).  ACT
applies tanh with the per-partition bias hp^T[:, b] + b_h + b_e and
writes bf16 energy; a [128 x BC] bf16 matmul against v4 (v in column b,
zeros elsewhere) reduces over g so batch b's scores land on PSUM
partition b; SBUF-to-SBUF DMA parks each batch's score row at partition
32*b, and that batch's fp32 softmax + output DMA run incrementally as
soon as its last token group finishes, hiding the tail inside the loop.
"""

import sys
from contextlib import ExitStack

import numpy as np

try:
    import concourse  # noqa: F401
except ImportError:  # pragma: no cover
    sys.path.insert(0, "/opt/trn_rl_repo")

import ml_dtypes

import concourse.tile as tile
from concourse import bacc, mybir
from concourse.bass import ts
from concourse.bass_utils import run_bass_kernel_spmd

H = 1024
T = 2048
B = 32
NCORES = 8
BC = B // NCORES          # batches per core
HC = H // 128             # h chunks
GC = H // 128             # g chunks
TOK = 512                 # tokens per group (one batch each)
NGRP_PER_B = T // TOK
NGRP = BC * NGRP_PER_B

F32 = mybir.dt.float32
BF16 = mybir.dt.bfloat16
AF = mybir.ActivationFunctionType
AX = mybir.AxisListType


def build_kernel_nc(reps=1):
    nc = bacc.Bacc(
        "TRN2",
        target_bir_lowering=False,
        debug=False,
        enable_asserts=False,
        num_devices=NCORES,
    )
    enc = nc.dram_tensor("enc", [T, BC, H], BF16, kind="ExternalInput").ap()
    weT = nc.dram_tensor("WeT", [H, H], BF16, kind="ExternalInput").ap()
    whT = nc.dram_tensor("WhT", [H, H], BF16, kind="ExternalInput").ap()
    hidT = nc.dram_tensor("hidT", [H, BC], BF16, kind="ExternalInput").ap()
    bsum = nc.dram_tensor("bsum", [H], F32, kind="ExternalInput").ap()
    v4d = nc.dram_tensor("v4", [128, GC * BC * BC], BF16, kind="ExternalInput").ap()
    out = nc.dram_tensor("out", [BC, T], F32, kind="ExternalOutput").ap()

    with tile.TileContext(nc) as tc:
        _kernel_body(tc, enc, weT, whT, hidT, bsum, v4d, out, reps=reps)
    nc.compile()
    return nc


def _kernel_body(tc, enc, weT, whT, hidT, bsum, v4d, out, reps=1):
    nc = tc.nc
    with ExitStack() as ctx:
        singles = ctx.enter_context(tc.tile_pool(name="singles", bufs=1))
        encT_pool = ctx.enter_context(tc.tile_pool(name="encT", bufs=3))
        energy_pool = ctx.enter_context(tc.tile_pool(name="energy", bufs=3))
        scrow_pool = ctx.enter_context(tc.tile_pool(name="scrow", bufs=2))
        ep_pool = ctx.enter_context(tc.tile_pool(name="epps", bufs=4, space="PSUM"))
        sc_pool = ctx.enter_context(tc.tile_pool(name="scps", bufs=2, space="PSUM"))

        # ---- persistent SBUF tensors -------------------------------------
        WeT = singles.tile([128, HC, H], BF16)     # WeT[h, hc, g] = W_e[g, 128*hc+h]
        v4 = singles.tile([128, GC, BC, BC], BF16)
        bias_all = singles.tile([128, GC, BC], F32)  # hp^T + b_h + b_e
        bsum_sb = singles.tile([128, GC], F32)     # (b_h + b_e) chunked
        # batch b's scores live on partition 32*b so per-batch softmax can
        # run as soon as that batch's groups finish (engine ops only accept
        # partition bases 0/32/64/96; DMA scatters the rows there)
        scores = singles.tile([128, T], F32)
        probs = singles.tile([128, T], F32)
        negmax = singles.tile([128, 1], F32)
        sums = singles.tile([128, 1], F32)
        rsum = singles.tile([128, 1], F32)

        # ---- stage 0: weight loads + hp + bias ---------------------------
        nc.sync.dma_start(
            out=WeT[:], in_=weT.rearrange("(hc p) g -> p hc g", p=128)
        )
        nc.sync.dma_start(out=v4[:], in_=v4d.rearrange("p (gc b c) -> p gc b c", b=BC, c=BC))
        nc.sync.dma_start(out=bsum_sb[:], in_=bsum.rearrange("(c p) -> p c", p=128))

        with tc.tile_pool(name="stage0", bufs=1) as wload, tc.tile_pool(
            name="hpps", bufs=2, space="PSUM"
        ) as hp_pool:
            WhT = wload.tile([128, HC, H], BF16)
            hidT_sb = wload.tile([128, HC, BC], BF16)
            nc.sync.dma_start(
                out=WhT[:], in_=whT.rearrange("(hc p) g -> p hc g", p=128)
            )
            nc.sync.dma_start(
                out=hidT_sb[:], in_=hidT.rearrange("(hc p) b -> p hc b", p=128)
            )
            # hp^T[g, b] accumulated over h chunks
            for gc in range(GC):
                hp_ps = hp_pool.tile([128, BC], F32)
                for hc in range(HC):
                    nc.tensor.matmul(
                        hp_ps[:],
                        WhT[:, hc, ts(gc, 128)],
                        hidT_sb[:, hc, :],
                        start=(hc == 0),
                        stop=(hc == HC - 1),
                    )
                nc.vector.tensor_scalar(
                    out=bias_all[:, gc, :],
                    in0=hp_ps[:],
                    scalar1=bsum_sb[:, gc : gc + 1],
                    scalar2=None,
                    op0=mybir.AluOpType.add,
                )

        # ---- main loop: 16 groups of 512 tokens --------------------------
        # Software-pipelined so the in-order PE queue never waits on ACT:
        #   iteration g emits: transpose-DMA(g+2), ep/sc chain(g)
        # with sc(gc-1) emitted after ep(gc) so tanh(gc-1) is long done.
        n_total = reps * NGRP

        def issue_load(grp):
            g = grp % NGRP
            b = g // NGRP_PER_B
            t0 = (g % NGRP_PER_B) * TOK
            encT = encT_pool.tile([128, HC, TOK], BF16, tag="encT")
            nc.sync.dma_start_transpose(
                out=encT[:], in_=enc[t0 : t0 + TOK, b, :]
            )
            return encT

        def softmax_b(b):
            r = slice(32 * b, 32 * b + 1)
            nc.vector.tensor_reduce(
                out=negmax[r], in_=scores[r, :], axis=AX.X,
                op=mybir.AluOpType.max, negate=True,
            )
            nc.scalar.activation(
                out=probs[r, :], in_=scores[r, :], func=AF.Exp,
                bias=negmax[r], scale=1.0, accum_out=sums[r],
            )
            nc.vector.reciprocal(out=rsum[r], in_=sums[r])
            nc.vector.tensor_scalar_mul(probs[r, :], probs[r, :], rsum[r])
            nc.sync.dma_start(out=out[b : b + 1, :], in_=probs[r, :])

        def flush_carry(c):
            c_sc_ps, c_gc, c_energy, c_b, c_t0 = c
            nc.tensor.matmul(
                c_sc_ps[:], v4[:, c_gc, c_b, :], c_energy[:],
                start=False, stop=True,
            )
            sc_sb = scrow_pool.tile([BC, TOK], F32)
            nc.vector.tensor_copy(sc_sb[:], c_sc_ps[:])
            nc.sync.dma_start(
                out=scores[32 * c_b : 32 * c_b + 1, c_t0 : c_t0 + TOK],
                in_=sc_sb[c_b : c_b + 1, :],
            )
            if c_t0 == T - TOK:
                softmax_b(c_b)

        encTs = [issue_load(0), issue_load(1)]
        carry = None  # deferred final sc-mm of the previous group

        for grp in range(n_total):
            g = grp % NGRP
            b = g // NGRP_PER_B
            t0 = (g % NGRP_PER_B) * TOK

            if grp + 2 < n_total:
                encTs.append(issue_load(grp + 2))
            encT_cur = encTs[grp]
            if carry is not None:
                flush_carry(carry)
                carry = None

            sc_ps = sc_pool.tile([BC, TOK], F32)
            pending = None
            for gc in range(GC):
                ep_ps = ep_pool.tile([128, TOK], F32)
                for hc in range(HC):
                    nc.tensor.matmul(
                        ep_ps[:],
                        WeT[:, hc, ts(gc, 128)],
                        encT_cur[:, hc, :],
                        start=(hc == 0),
                        stop=(hc == HC - 1),
                    )
                if pending is not None:
                    pc, penergy = pending
                    nc.tensor.matmul(
                        sc_ps[:], v4[:, pc, b, :], penergy[:],
                        start=(pc == 0), stop=False,
                    )
                energy = energy_pool.tile([128, TOK], BF16)
                nc.scalar.activation(
                    out=energy[:],
                    in_=ep_ps[:],
                    func=AF.Tanh,
                    bias=bias_all[:, gc, b : b + 1],
                    scale=1.0,
                )
                pending = (gc, energy)
            pc, penergy = pending
            carry = (sc_ps, pc, penergy, b, t0)
            encTs[grp] = None  # release reference

        flush_carry(carry)


_NC_CACHE = None


def _get_nc():
    global _NC_CACHE
    if _NC_CACHE is None:
        _NC_CACHE = build_kernel_nc()
    return _NC_CACHE


def make_in_maps(hidden, encoder_outputs, W_h, b_h, W_e, b_e, v):
    hidden = np.asarray(hidden, dtype=np.float32)
    enc = np.asarray(encoder_outputs, dtype=np.float32)
    W_h = np.asarray(W_h, dtype=np.float32)
    W_e = np.asarray(W_e, dtype=np.float32)
    b_h = np.asarray(b_h, dtype=np.float32)
    b_e = np.asarray(b_e, dtype=np.float32)
    v = np.asarray(v, dtype=np.float32)

    enc_bf = enc.astype(ml_dtypes.bfloat16)
    weT = np.ascontiguousarray(W_e.T).astype(ml_dtypes.bfloat16)
    whT = np.ascontiguousarray(W_h.T).astype(ml_dtypes.bfloat16)
    bsum = np.ascontiguousarray(b_h + b_e)
    hid0 = hidden.reshape(B, H)

    # v4[p, gc, b, b'] = v[gc*128 + p] if b == b' else 0
    v4 = np.zeros((128, GC, BC, BC), dtype=np.float32)
    vc = v.reshape(GC, 128)  # [gc, p]
    for b in range(BC):
        v4[:, :, b, b] = vc.T
    v4 = np.ascontiguousarray(v4.reshape(128, GC * BC * BC)).astype(
        ml_dtypes.bfloat16
    )

    in_maps = []
    for c in range(NCORES):
        hidT = np.ascontiguousarray(
            hid0[c * BC : (c + 1) * BC, :].T
        ).astype(ml_dtypes.bfloat16)
        in_maps.append(
            {
                "enc": np.ascontiguousarray(enc_bf[:, c * BC : (c + 1) * BC, :]),
                "WeT": weT,
                "WhT": whT,
                "hidT": hidT,
                "bsum": bsum,
                "v4": v4,
            }
        )
    return in_maps


def kernel(hidden, encoder_outputs, W_h, b_h, W_e, b_e, v):
    nc = _get_nc()
    in_maps = make_in_maps(hidden, encoder_outputs, W_h, b_h, W_e, b_e, v)
    res = run_bass_kernel_spmd(nc, in_maps, list(range(NCORES)))
    full = np.concatenate([res.results[c]["out"] for c in range(NCORES)], axis=0)
    return full[:, None, :].astype(np.float32)


# revision 3
# speedup vs baseline: 1.3507x; 1.2014x over previous
"""Bahdanau attention on 8 Trainium2 cores (Bass/Tile), data-parallel over B.

reference (per batch b, all shapes full):
    hp  = hidden[0] @ W_h.T + b_h                    # (B, H)
    ep  = einsum('tbh,gh->btg', enc, W_e) + b_e      # (B, T, H)
    en  = tanh(hp[:, None, :] + ep)                  # (B, T, H)
    sc  = en @ v                                     # (B, T)
    out = softmax(sc, -1)[:, None, :]                # (B, 1, T)

Sharding: B=32 split 4-per-core across 8 cores; weights replicated.

v3 layout: host stages enc as bf16 (halves HBM traffic) and ships the
weights pre-transposed (W^T, layout-only) in bf16.  Each 1024-token
supergroup of one batch arrives in SBUF already transposed to
[h=128, hc, tok] via XBAR dma_start_transpose straight from DRAM, so
the PE runs nothing but ep/score matmuls (bf16, fp32 PSUM).  The two
512-token halves of a supergroup share each stationary weight load
back-to-back and accumulate into the two banks of one [128,2,512] PSUM
tile; ACT then applies tanh over both halves in one instruction (same
per-partition bias hp^T[:, b] + b_h + b_e) writing bf16 energy; [128xBC]
bf16 matmuls against v4 (v in column b, zeros elsewhere) reduce over g
so batch b's scores land on PSUM partition b.  SBUF-to-SBUF DMA parks
score rows at partition 32*b; each batch's softmax is split into
chunked ACT exp instructions interleaved into the next supergroup's
tanh stream so the single-lane exp never blocks the ACT FIFO.
"""

import sys
from contextlib import ExitStack

import numpy as np

try:
    import concourse  # noqa: F401
except ImportError:  # pragma: no cover
    sys.path.insert(0, "/opt/trn_rl_repo")

import ml_dtypes

import concourse.tile as tile
from concourse import bacc, mybir
from concourse.bass import ts
from concourse.bass_utils import run_bass_kernel_spmd

H = 1024
T = 2048
B = 32
NCORES = 8
BC = B // NCORES          # batches per core
HC = H // 128             # h chunks
GC = H // 128             # g chunks
TOK = 512                 # tokens per matmul / PSUM bank
SG = 2 * TOK              # tokens per supergroup (one batch each)
NSG_PER_B = T // SG
NSG = BC * NSG_PER_B
NEXPC = 4                 # softmax exp chunks per batch

F32 = mybir.dt.float32
BF16 = mybir.dt.bfloat16
AF = mybir.ActivationFunctionType
AX = mybir.AxisListType


def build_kernel_nc(reps=1):
    nc = bacc.Bacc(
        "TRN2",
        target_bir_lowering=False,
        debug=False,
        enable_asserts=False,
        num_devices=NCORES,
    )
    enc = nc.dram_tensor("enc", [T, BC, H], BF16, kind="ExternalInput").ap()
    weT = nc.dram_tensor("WeT", [H, H], BF16, kind="ExternalInput").ap()
    whT = nc.dram_tensor("WhT", [H, H], BF16, kind="ExternalInput").ap()
    hidT = nc.dram_tensor("hidT", [H, BC], BF16, kind="ExternalInput").ap()
    bsum = nc.dram_tensor("bsum", [H], F32, kind="ExternalInput").ap()
    v4d = nc.dram_tensor("v4", [128, GC * BC * BC], BF16, kind="ExternalInput").ap()
    out = nc.dram_tensor("out", [BC, T], F32, kind="ExternalOutput").ap()

    with tile.TileContext(nc) as tc:
        _kernel_body(tc, enc, weT, whT, hidT, bsum, v4d, out, reps=reps)
    nc.compile()
    return nc


def _kernel_body(tc, enc, weT, whT, hidT, bsum, v4d, out, reps=1):
    nc = tc.nc
    with ExitStack() as ctx:
        singles = ctx.enter_context(tc.tile_pool(name="singles", bufs=1))

        # ---- persistent SBUF tensors -------------------------------------
        WeT = singles.tile([128, HC, H], BF16)     # WeT[h, hc, g] = W_e[g, 128*hc+h]
        v4 = singles.tile([128, GC, BC, BC], BF16)
        bias_all = singles.tile([128, GC, BC], F32)  # hp^T + b_h + b_e
        bsum_sb = singles.tile([128, GC], F32)     # (b_h + b_e) chunked
        # batch b's scores live on partition 32*b so per-batch softmax can
        # run as soon as that batch's groups finish (engine ops only accept
        # partition bases 0/32/64/96; DMA scatters the rows there)
        scores = singles.tile([128, T], F32)
        probs = singles.tile([128, T], F32)
        negmax = singles.tile([128, 1], F32)
        sums = singles.tile([128, NEXPC], F32)
        stot = singles.tile([128, 1], F32)
        rsum = singles.tile([128, 1], F32)

        # ---- stage 0: weight loads + hp + bias ---------------------------
        nc.sync.dma_start(
            out=WeT[:], in_=weT.rearrange("(hc p) g -> p hc g", p=128)
        )
        nc.sync.dma_start(
            out=v4[:], in_=v4d.rearrange("p (gc b c) -> p gc b c", b=BC, c=BC)
        )
        nc.sync.dma_start(out=bsum_sb[:], in_=bsum.rearrange("(c p) -> p c", p=128))

        with tc.tile_pool(name="stage0", bufs=1) as wload, tc.tile_pool(
            name="hpps", bufs=2, space="PSUM"
        ) as hp_pool:
            WhT = wload.tile([128, HC, H], BF16)
            hidT_sb = wload.tile([128, HC, BC], BF16)
            nc.sync.dma_start(
                out=WhT[:], in_=whT.rearrange("(hc p) g -> p hc g", p=128)
            )
            nc.sync.dma_start(
                out=hidT_sb[:], in_=hidT.rearrange("(hc p) b -> p hc b", p=128)
            )
            # hp^T[g, b] accumulated over h chunks
            for gc in range(GC):
                hp_ps = hp_pool.tile([128, BC], F32)
                for hc in range(HC):
                    nc.tensor.matmul(
                        hp_ps[:],
                        WhT[:, hc, ts(gc, 128)],
                        hidT_sb[:, hc, :],
                        start=(hc == 0),
                        stop=(hc == HC - 1),
                    )
                nc.vector.tensor_scalar(
                    out=bias_all[:, gc, :],
                    in0=hp_ps[:],
                    scalar1=bsum_sb[:, gc : gc + 1],
                    scalar2=None,
                    op0=mybir.AluOpType.add,
                )

        # ---- main loop: 8 supergroups of 1024 tokens ---------------------
        # Iteration s emits: transpose-DMA(s+2), then per gc the 16 ep
        # matmuls (two 512-halves, shared stationary), the two deferred
        # sc matmuls of gc-1, one fused tanh, and (when a batch just
        # finished) one interleaved softmax-exp chunk.
        encT_pool = ctx.enter_context(tc.tile_pool(name="encT", bufs=3))
        energy_pool = ctx.enter_context(tc.tile_pool(name="energy", bufs=3))
        scrow_pool = ctx.enter_context(tc.tile_pool(name="scrow", bufs=2))
        ep_pool = ctx.enter_context(tc.tile_pool(name="epps", bufs=2, space="PSUM"))
        sc_pool = ctx.enter_context(tc.tile_pool(name="scps", bufs=2, space="PSUM"))

        n_total = reps * NSG

        def issue_load(sg):
            s = sg % NSG
            b = s // NSG_PER_B
            t0 = (s % NSG_PER_B) * SG
            encT = encT_pool.tile([128, HC, 2, TOK], BF16, tag="encT")
            for half in range(2):
                th = t0 + half * TOK
                nc.sync.dma_start_transpose(
                    out=encT[:, :, half, :], in_=enc[th : th + TOK, b, :]
                )
            return encT

        def softmax_head(b):
            # negmax must precede the interleaved exp chunks
            r = slice(32 * b, 32 * b + 1)
            nc.vector.tensor_reduce(
                out=negmax[r], in_=scores[r, :], axis=AX.X,
                op=mybir.AluOpType.max, negate=True,
            )

        def softmax_chunk(b, c):
            r = slice(32 * b, 32 * b + 1)
            w = T // NEXPC
            nc.scalar.activation(
                out=probs[r, ts(c, w)], in_=scores[r, ts(c, w)], func=AF.Exp,
                bias=negmax[r], scale=1.0, accum_out=sums[r, c : c + 1],
            )

        def softmax_tail(b):
            r = slice(32 * b, 32 * b + 1)
            nc.vector.tensor_reduce(
                out=stot[r], in_=sums[r, :], axis=AX.X, op=mybir.AluOpType.add,
            )
            nc.vector.reciprocal(out=rsum[r], in_=stot[r])
            nc.vector.tensor_scalar_mul(probs[r, :], probs[r, :], rsum[r])
            nc.sync.dma_start(out=out[b : b + 1, :], in_=probs[r, :])

        def flush_carry(c):
            c_sc_ps, c_gc, c_energy, c_b, c_t0 = c
            for half in range(2):
                nc.tensor.matmul(
                    c_sc_ps[:, half, :], v4[:, c_gc, c_b, :],
                    c_energy[:, half, :], start=False, stop=True,
                )
            sc_sb = scrow_pool.tile([BC, 2, TOK], F32)
            nc.vector.tensor_copy(sc_sb[:], c_sc_ps[:])
            nc.sync.dma_start(
                out=scores[32 * c_b : 32 * c_b + 1, c_t0 : c_t0 + SG],
                in_=sc_sb[c_b : c_b + 1, :, :],
            )
            return c_b if c_t0 == T - SG else None

        encTs = [issue_load(0), issue_load(1)]
        carry = None          # deferred final sc-mms of the previous group
        sm_batch = None       # batch whose softmax-exp chunks are pending

        for sg in range(n_total):
            s = sg % NSG
            b = s // NSG_PER_B
            t0 = (s % NSG_PER_B) * SG

            if sg + 2 < n_total:
                encTs.append(issue_load(sg + 2))
            encT_cur = encTs[sg]
            finished = None
            if carry is not None:
                finished = flush_carry(carry)
                carry = None
            if finished is not None:
                softmax_head(finished)
                sm_batch = finished

            sc_ps = sc_pool.tile([BC, 2, TOK], F32)
            pending = None
            for gc in range(GC):
                ep_ps = ep_pool.tile([128, 2, TOK], F32)
                for hc in range(HC):
                    for half in range(2):
                        nc.tensor.matmul(
                            ep_ps[:, half, :],
                            WeT[:, hc, ts(gc, 128)],
                            encT_cur[:, hc, half, :],
                            start=(hc == 0),
                            stop=(hc == HC - 1),
                        )
                if pending is not None:
                    pc, penergy = pending
                    for half in range(2):
                        nc.tensor.matmul(
                            sc_ps[:, half, :], v4[:, pc, b, :],
                            penergy[:, half, :], start=(pc == 0), stop=False,
                        )
                energy = energy_pool.tile([128, 2, TOK], BF16)
                nc.scalar.activation(
                    out=energy[:],
                    in_=ep_ps[:],
                    func=AF.Tanh,
                    bias=bias_all[:, gc, b : b + 1],
                    scale=1.0,
                )
                if sm_batch is not None and gc % 2 == 1:
                    softmax_chunk(sm_batch, gc // 2)
                    if gc == GC - 1:
                        softmax_tail(sm_batch)
                        sm_batch = None
                pending = (gc, energy)
            pc, penergy = pending
            carry = (sc_ps, pc, penergy, b, t0)
            encTs[sg] = None  # release reference

        finished = flush_carry(carry)
        if finished is not None:
            softmax_head(finished)
            for c in range(NEXPC):
                softmax_chunk(finished, c)
            softmax_tail(finished)


_NC_CACHE = None


def _get_nc():
    global _NC_CACHE
    if _NC_CACHE is None:
        _NC_CACHE = build_kernel_nc()
    return _NC_CACHE


def make_in_maps(hidden, encoder_outputs, W_h, b_h, W_e, b_e, v):
    hidden = np.asarray(hidden, dtype=np.float32)
    enc = np.asarray(encoder_outputs, dtype=np.float32)
    W_h = np.asarray(W_h, dtype=np.float32)
    W_e = np.asarray(W_e, dtype=np.float32)
    b_h = np.asarray(b_h, dtype=np.float32)
    b_e = np.asarray(b_e, dtype=np.float32)
    v = np.asarray(v, dtype=np.float32)

    enc_bf = enc.astype(ml_dtypes.bfloat16)
    weT = np.ascontiguousarray(W_e.T).astype(ml_dtypes.bfloat16)
    whT = np.ascontiguousarray(W_h.T).astype(ml_dtypes.bfloat16)
    bsum = np.ascontiguousarray(b_h + b_e)
    hid0 = hidden.reshape(B, H)

    # v4[p, gc, b, b'] = v[gc*128 + p] if b == b' else 0
    v4 = np.zeros((128, GC, BC, BC), dtype=np.float32)
    vc = v.reshape(GC, 128)  # [gc, p]
    for b in range(BC):
        v4[:, :, b, b] = vc.T
    v4 = np.ascontiguousarray(v4.reshape(128, GC * BC * BC)).astype(
        ml_dtypes.bfloat16
    )

    in_maps = []
    for c in range(NCORES):
        hidT = np.ascontiguousarray(
            hid0[c * BC : (c + 1) * BC, :].T
        ).astype(ml_dtypes.bfloat16)
        in_maps.append(
            {
                "enc": np.ascontiguousarray(enc_bf[:, c * BC : (c + 1) * BC, :]),
                "WeT": weT,
                "WhT": whT,
                "hidT": hidT,
                "bsum": bsum,
                "v4": v4,
            }
        )
    return in_maps


def kernel(hidden, encoder_outputs, W_h, b_h, W_e, b_e, v):
    nc = _get_nc()
    in_maps = make_in_maps(hidden, encoder_outputs, W_h, b_h, W_e, b_e, v)
    res = run_bass_kernel_spmd(nc, in_maps, list(range(NCORES)))
    full = np.concatenate([res.results[c]["out"] for c in range(NCORES)], axis=0)
    return full[:, None, :].astype(np.float32)


# revision 8
# speedup vs baseline: 1.4165x; 1.0488x over previous
"""Bahdanau attention on 8 Trainium2 cores (Bass/Tile), data-parallel over B.

reference (per batch b, all shapes full):
    hp  = hidden[0] @ W_h.T + b_h                    # (B, H)
    ep  = einsum('tbh,gh->btg', enc, W_e) + b_e      # (B, T, H)
    en  = tanh(hp[:, None, :] + ep)                  # (B, T, H)
    sc  = en @ v                                     # (B, T)
    out = softmax(sc, -1)[:, None, :]                # (B, 1, T)

Sharding: B=32 split 4-per-core across 8 cores; weights replicated.

v3 layout: host stages enc as bf16 (halves HBM traffic) and ships the
weights pre-transposed (W^T, layout-only) in bf16.  Each 1024-token
supergroup of one batch arrives in SBUF already transposed to
[h=128, hc, tok] via XBAR dma_start_transpose straight from DRAM, so
the PE runs nothing but ep/score matmuls (bf16, fp32 PSUM).  The two
512-token halves of a supergroup share each stationary weight load
back-to-back and accumulate into the two banks of one [128,2,512] PSUM
tile; ACT then applies tanh over both halves in one instruction (same
per-partition bias hp^T[:, b] + b_h + b_e) writing bf16 energy; [128xBC]
bf16 matmuls against v4 (v in column b, zeros elsewhere) reduce over g
so batch b's scores land on PSUM partition b.  SBUF-to-SBUF DMA parks
score rows at partition 32*b; each batch's softmax is split into
chunked ACT exp instructions interleaved into the next supergroup's
tanh stream so the single-lane exp never blocks the ACT FIFO.
"""

import sys
from contextlib import ExitStack

import numpy as np

try:
    import concourse  # noqa: F401
except ImportError:  # pragma: no cover
    sys.path.insert(0, "/opt/trn_rl_repo")

import ml_dtypes

import concourse.tile as tile
from concourse import bacc, mybir
from concourse.bass import ts
from concourse.bass_utils import run_bass_kernel_spmd

H = 1024
T = 2048
B = 32
NCORES = 8
BC = B // NCORES          # batches per core
HC = H // 128             # h chunks
GC = H // 128             # g chunks
TOK = 512                 # tokens per matmul / PSUM bank
SG = 2 * TOK              # tokens per supergroup (one batch each)
NSG_PER_B = T // SG
NSG = BC * NSG_PER_B
NEXPC = 4                 # softmax exp chunks per batch

F32 = mybir.dt.float32
BF16 = mybir.dt.bfloat16
AF = mybir.ActivationFunctionType
AX = mybir.AxisListType


def build_kernel_nc(reps=1):
    nc = bacc.Bacc(
        "TRN2",
        target_bir_lowering=False,
        debug=False,
        enable_asserts=False,
        num_devices=NCORES,
    )
    enc = nc.dram_tensor("enc", [T, BC, H], BF16, kind="ExternalInput").ap()
    weT = nc.dram_tensor("WeT", [H, H], BF16, kind="ExternalInput").ap()
    whT = nc.dram_tensor("WhT", [H, H], BF16, kind="ExternalInput").ap()
    hidT = nc.dram_tensor("hidT", [H, BC], BF16, kind="ExternalInput").ap()
    bsum = nc.dram_tensor("bsum", [H], F32, kind="ExternalInput").ap()
    v4d = nc.dram_tensor("v4", [128, GC * 128], BF16, kind="ExternalInput").ap()
    out = nc.dram_tensor("out", [BC, T], F32, kind="ExternalOutput").ap()

    with tile.TileContext(nc) as tc:
        _kernel_body(tc, enc, weT, whT, hidT, bsum, v4d, out, reps=reps)
    nc.compile()
    return nc


def _kernel_body(tc, enc, weT, whT, hidT, bsum, v4d, out, reps=1):
    nc = tc.nc
    with ExitStack() as ctx:
        singles = ctx.enter_context(tc.tile_pool(name="singles", bufs=1))

        # ---- persistent SBUF tensors -------------------------------------
        WeT = singles.tile([128, HC, H], BF16)     # WeT[h, hc, g] = W_e[g, 128*hc+h]
        # v4[p, gc, 32*b] = v[gc*128+p] for every b (else 0): the score
        # matmul against energy then lands batch data on PSUM partitions
        # 0/32/64/96 simultaneously; we only read row 32*b for batch b.
        v4 = singles.tile([128, GC, 128], BF16)
        bias_all = singles.tile([128, GC, BC], F32)  # hp^T + b_h + b_e
        bsum_sb = singles.tile([128, GC], F32)     # (b_h + b_e) chunked
        # batch b's scores live on partition 32*b so per-batch softmax can
        # run as soon as that batch's groups finish (engine ops only accept
        # partition bases 0/32/64/96; DMA scatters the rows there)
        scores = singles.tile([128, T], F32)
        probs = singles.tile([128, T], F32)
        negmax = singles.tile([128, 1], F32)
        sums = singles.tile([128, NEXPC], F32)
        stot = singles.tile([128, 1], F32)
        rsum = singles.tile([128, 1], F32)

        # ---- stage 0: weight loads + hp + bias ---------------------------
        nc.sync.dma_start(
            out=WeT[:], in_=weT.rearrange("(hc p) g -> p hc g", p=128)
        )
        nc.sync.dma_start(
            out=v4[:], in_=v4d.rearrange("p (gc c) -> p gc c", c=128)
        )
        nc.sync.dma_start(out=bsum_sb[:], in_=bsum.rearrange("(c p) -> p c", p=128))

        with tc.tile_pool(name="stage0", bufs=1) as wload, tc.tile_pool(
            name="hpps", bufs=2, space="PSUM"
        ) as hp_pool:
            WhT = wload.tile([128, HC, H], BF16)
            hidT_sb = wload.tile([128, HC, BC], BF16)
            nc.sync.dma_start(
                out=WhT[:], in_=whT.rearrange("(hc p) g -> p hc g", p=128)
            )
            nc.sync.dma_start(
                out=hidT_sb[:], in_=hidT.rearrange("(hc p) b -> p hc b", p=128)
            )
            # hp^T[g, b] accumulated over h chunks
            for gc in range(GC):
                hp_ps = hp_pool.tile([128, BC], F32)
                for hc in range(HC):
                    nc.tensor.matmul(
                        hp_ps[:],
                        WhT[:, hc, ts(gc, 128)],
                        hidT_sb[:, hc, :],
                        start=(hc == 0),
                        stop=(hc == HC - 1),
                    )
                nc.vector.tensor_scalar(
                    out=bias_all[:, gc, :],
                    in0=hp_ps[:],
                    scalar1=bsum_sb[:, gc : gc + 1],
                    scalar2=None,
                    op0=mybir.AluOpType.add,
                )

        # ---- main loop: 8 supergroups of 1024 tokens ---------------------
        # Iteration s emits: transpose-DMA(s+2), then per gc the 16 ep
        # matmuls (two 512-halves, shared stationary), the two deferred
        # sc matmuls of gc-1, one fused tanh, and (when a batch just
        # finished) one interleaved softmax-exp chunk.
        encT_pool = ctx.enter_context(tc.tile_pool(name="encT", bufs=3))
        energy_pool = ctx.enter_context(tc.tile_pool(name="energy", bufs=3))
        ep_pool = ctx.enter_context(tc.tile_pool(name="epps", bufs=2, space="PSUM"))
        sc_pool = ctx.enter_context(tc.tile_pool(name="scps", bufs=2, space="PSUM"))

        n_total = reps * NSG

        def issue_load(sg):
            s = sg % NSG
            b = s // NSG_PER_B
            t0 = (s % NSG_PER_B) * SG
            encT = encT_pool.tile([128, HC, 2, TOK], BF16, tag="encT")
            for half in range(2):
                th = t0 + half * TOK
                nc.sync.dma_start_transpose(
                    out=encT[:, :, half, :], in_=enc[th : th + TOK, b, :]
                )
            return encT

        def softmax_head(b):
            # negmax must precede the interleaved exp chunks
            r = slice(32 * b, 32 * b + 1)
            nc.vector.tensor_reduce(
                out=negmax[r], in_=scores[r, :], axis=AX.X,
                op=mybir.AluOpType.max, negate=True,
            )

        def softmax_chunk(b, c):
            r = slice(32 * b, 32 * b + 1)
            w = T // NEXPC
            nc.scalar.activation(
                out=probs[r, ts(c, w)], in_=scores[r, ts(c, w)], func=AF.Exp,
                bias=negmax[r], scale=1.0, accum_out=sums[r, c : c + 1],
            )

        def softmax_tail(b):
            r = slice(32 * b, 32 * b + 1)
            nc.vector.tensor_reduce(
                out=stot[r], in_=sums[r, :], axis=AX.X, op=mybir.AluOpType.add,
            )
            nc.vector.reciprocal(out=rsum[r], in_=stot[r])
            nc.vector.tensor_scalar_mul(probs[r, :], probs[r, :], rsum[r])
            nc.sync.dma_start(out=out[b : b + 1, :], in_=probs[r, :])

        def flush_carry(c):
            c_sc_ps, c_gc, c_energy, c_b, c_t0 = c
            for half in range(2):
                nc.tensor.matmul(
                    c_sc_ps[:, half, :], v4[:, c_gc, :],
                    c_energy[:, half, :], start=False, stop=True,
                )
            r = slice(32 * c_b, 32 * c_b + 1)
            nc.vector.tensor_copy(
                scores[r, c_t0 : c_t0 + SG], c_sc_ps[r, :, :]
            )
            return c_b if c_t0 == T - SG else None

        encTs = [issue_load(0), issue_load(1)]
        carry = None          # deferred final sc-mms of the previous group
        sm_batch = None       # batch whose softmax-exp chunks are pending

        for sg in range(n_total):
            s = sg % NSG
            b = s // NSG_PER_B
            t0 = (s % NSG_PER_B) * SG

            if sg + 2 < n_total:
                encTs.append(issue_load(sg + 2))
            encT_cur = encTs[sg]
            finished = None
            if carry is not None:
                finished = flush_carry(carry)
                carry = None
            if finished is not None:
                softmax_head(finished)
                sm_batch = finished

            sc_ps = sc_pool.tile([128, 2, TOK], F32)
            pending = None
            for gc in range(GC):
                ep_ps = ep_pool.tile([128, 2, TOK], F32)
                for hc in range(HC):
                    for half in range(2):
                        nc.tensor.matmul(
                            ep_ps[:, half, :],
                            WeT[:, hc, ts(gc, 128)],
                            encT_cur[:, hc, half, :],
                            start=(hc == 0),
                            stop=(hc == HC - 1),
                        )
                if pending is not None:
                    pc, penergy = pending
                    for half in range(2):
                        nc.tensor.matmul(
                            sc_ps[:, half, :], v4[:, pc, :],
                            penergy[:, half, :], start=(pc == 0), stop=False,
                        )
                energy = energy_pool.tile([128, 2, TOK], BF16)
                nc.scalar.activation(
                    out=energy[:],
                    in_=ep_ps[:],
                    func=AF.Tanh,
                    bias=bias_all[:, gc, b : b + 1],
                    scale=1.0,
                )
                if sm_batch is not None and gc % 2 == 1:
                    softmax_chunk(sm_batch, gc // 2)
                    if gc == GC - 1:
                        softmax_tail(sm_batch)
                        sm_batch = None
                pending = (gc, energy)
            pc, penergy = pending
            carry = (sc_ps, pc, penergy, b, t0)
            encTs[sg] = None  # release reference

        finished = flush_carry(carry)
        if finished is not None:
            softmax_head(finished)
            for c in range(NEXPC):
                softmax_chunk(finished, c)
            softmax_tail(finished)


_NC_CACHE = None


def _get_nc():
    global _NC_CACHE
    if _NC_CACHE is None:
        _NC_CACHE = build_kernel_nc()
    return _NC_CACHE


def make_in_maps(hidden, encoder_outputs, W_h, b_h, W_e, b_e, v):
    hidden = np.asarray(hidden, dtype=np.float32)
    enc = np.asarray(encoder_outputs, dtype=np.float32)
    W_h = np.asarray(W_h, dtype=np.float32)
    W_e = np.asarray(W_e, dtype=np.float32)
    b_h = np.asarray(b_h, dtype=np.float32)
    b_e = np.asarray(b_e, dtype=np.float32)
    v = np.asarray(v, dtype=np.float32)

    enc_bf = enc.astype(ml_dtypes.bfloat16)
    weT = np.ascontiguousarray(W_e.T).astype(ml_dtypes.bfloat16)
    whT = np.ascontiguousarray(W_h.T).astype(ml_dtypes.bfloat16)
    bsum = np.ascontiguousarray(b_h + b_e)
    hid0 = hidden.reshape(B, H)

    # v4[p, gc, 32*b] = v[gc*128 + p] for every b (else 0)
    v4 = np.zeros((128, GC, 128), dtype=np.float32)
    vc = v.reshape(GC, 128)  # [gc, p]
    for b in range(BC):
        v4[:, :, 32 * b] = vc.T
    v4 = np.ascontiguousarray(v4.reshape(128, GC * 128)).astype(
        ml_dtypes.bfloat16
    )

    in_maps = []
    for c in range(NCORES):
        hidT = np.ascontiguousarray(
            hid0[c * BC : (c + 1) * BC, :].T
        ).astype(ml_dtypes.bfloat16)
        in_maps.append(
            {
                "enc": np.ascontiguousarray(enc_bf[:, c * BC : (c + 1) * BC, :]),
                "WeT": weT,
                "WhT": whT,
                "hidT": hidT,
                "bsum": bsum,
                "v4": v4,
            }
        )
    return in_maps


def kernel(hidden, encoder_outputs, W_h, b_h, W_e, b_e, v):
    nc = _get_nc()
    in_maps = make_in_maps(hidden, encoder_outputs, W_h, b_h, W_e, b_e, v)
    res = run_bass_kernel_spmd(nc, in_maps, list(range(NCORES)))
    full = np.concatenate([res.results[c]["out"] for c in range(NCORES)], axis=0)
    return full[:, None, :].astype(np.float32)
